# revision 1
# baseline (speedup 1.0000x reference)
"""Trainium2 Bass kernel for mixed Gaussian/Gabor splat rasterization.

Problem: render 3072 plain 2D gaussians + 1024 gabor-modulated gaussians
(G=4 cosine carriers each) densely into a [1,3,256,256] image, clamp to [0,1].

Strategy (8 NeuronCores, SPMD, no collectives):
  - Shard PIXELS: core k owns image rows [32k, 32k+32). Within a core, pixels
    are processed in 8 column-blocks ("superblocks") of 32x32 pixels, each
    with its own centered coordinate frame (|xc'|,|yc'| <= 16). Small
    coordinates keep the rank-5 sigma matmul well-conditioned under the PE's
    reduced-precision float32r format (~2^-17 relative).
  - sigma(i,px) = G5[:,i]^T . P5[:,px] + w5(i):  P5 = [xc'^2, xc'yc', yc'^2,
    xc', yc'] per-superblock basis, K=5 float32r matmuls into PSUM. The
    constant term w5 (big for distant gaussians) never enters the matmul: it
    rides the ScalarEngine Exp bias in full fp32:  w = Exp(-sigma5 - w5).
  - gabor phase: t = (fx*xc' + fy*yc')/2pi via K=2 f32r matmul; the constant
    (TOFF - (fx*xci+fy*yci)/2pi + shifts) rides the DVE op:
    u0 = (t + fbias) mod 1.0, then cos = Sin(2pi*u0 - pi) on ACT, with all
    4 carriers' u0 packed into one [128, 4096] tile so one Sin call serves
    a whole chunk (amortizes the ~293ns ACT instruction overhead).
  - carrier sum mod = sum_g wg*cos_g: PE matmuls with diag(wg) weights
    (diag built on-device as identity * wg_broadcast), PSUM-accumulated.
  - image img[3,px] += colors[128,3]^T @ W[128,px]: K=128 bf16 matmuls
    chained over all 32 chunks in one PSUM accumulation group per block.
  - clamp on DVE (max 0, min 1), DMA out per superblock; host reassembles
    column blocks into rows (pure indexing).
Per-superblock ACT ordering batches all Sin then all Exp (sin and exp live
in different activation-table sets; interleaving would reload tables).
Per-superblock sigma weights w3',w4',w5' are recomputed from global planes
with ~20 small DVE ops and re-transposed (PE) per block, overlapping the
main-loop compute.
"""

import math
import numpy as np

try:
    import concourse.bass as bass
except ImportError:
    import sys
    sys.path.insert(0, "/opt/trn_rl_repo")
    import concourse.bass as bass

import concourse.tile as tile
from concourse import bacc, mybir
from concourse.bass_utils import run_bass_kernel_spmd

F32 = mybir.dt.float32
F32R = mybir.dt.float32r
BF16 = mybir.dt.bfloat16
OP = mybir.AluOpType
AF = mybir.ActivationFunctionType

H = 256
W = 256
NL = 3072
NH = 1024
G = 4
NCORES = 8
ROWS = H // NCORES          # 32 rows per core
PX = ROWS * W               # 8192 pixels per core
SB = 1024                   # superblock = 32 cols x 32 rows
NSB = PX // SB              # 8 column blocks
CB = 32                     # columns per superblock
NLC = NL // 128             # 24
NHC = NH // 128             # 8
NCH = NLC + NHC             # 32
INV2PI = 1.0 / (2.0 * math.pi)
TOFF = 16.75                # 0.25 (cos->sin shift) + 16.5 (positivity)

_CACHE = {}


def _x0(sb):
    # x-center of column block sb (in centered image coords)
    return 32.0 * sb - 112.0


def _build_program():
    nc = bacc.Bacc("TRN2", target_bir_lowering=False, debug=False)

    lmu = nc.declare_dram_parameter("lmu", [NL, 2], F32, isOutput=False)
    lch = nc.declare_dram_parameter("lch", [NL, 3], F32, isOutput=False)
    lft = nc.declare_dram_parameter("lft", [NL, 3], F32, isOutput=False)
    lop = nc.declare_dram_parameter("lop", [NL, 1], F32, isOutput=False)
    hmu = nc.declare_dram_parameter("hmu", [NH, 2], F32, isOutput=False)
    hch = nc.declare_dram_parameter("hch", [NH, 3], F32, isOutput=False)
    hft = nc.declare_dram_parameter("hft", [NH, 3], F32, isOutput=False)
    hop = nc.declare_dram_parameter("hop", [NH, 1], F32, isOutput=False)
    gfx = nc.declare_dram_parameter("gfx", [NH, G], F32, isOutput=False)
    gfy = nc.declare_dram_parameter("gfy", [NH, G], F32, isOutput=False)
    gwg = nc.declare_dram_parameter("gwg", [NH, G], F32, isOutput=False)
    basis = nc.declare_dram_parameter("basis", [13, PX], F32R, isOutput=False)
    basisq = nc.declare_dram_parameter("basisq", [6, PX], F32R, isOutput=False)
    ident = nc.declare_dram_parameter("ident", [128, 128], F32, isOutput=False)
    ycen = nc.declare_dram_parameter("ycen", [128, 1], F32, isOutput=False)
    out_ext = nc.declare_dram_parameter("out", [3, PX], F32, isOutput=True)

    with tile.TileContext(nc, pool_alloc_mode="queue") as tc:
        with tc.tile_pool(name="singles", bufs=1) as singles:
            _body(nc, tc, singles, lmu, lch, lft, lop, hmu, hch, hft, hop,
                  gfx, gfy, gwg, basis, basisq, ident, ycen, out_ext)
    nc.finalize()
    return nc


def _body(nc, tc, singles, lmu, lch, lft, lop, hmu, hch, hft, hop,
          gfx, gfy, gwg, basis, basisq, ident, ycen, out_ext):
    V = nc.vector
    S = nc.scalar
    T = nc.tensor

    # ---------------- persistent SBUF tensors ----------------
    basis_sb = singles.tile([13, PX], F32R)
    basisq_sb = singles.tile([6, PX], F32R)
    ident_d = singles.tile([128, 128], F32)
    nc.gpsimd.dma_start(out=ident_d, in_=ident[:])
    ident_sb = singles.tile([128, 128], F32)
    V.tensor_copy(out=ident_sb, in_=ident_d)
    ycen_sb = singles.tile([128, 1], F32)
    nc.gpsimd.dma_start(out=ycen_sb, in_=ycen[:])
    ycen2_sb = singles.tile([128, 1], F32)
    V.tensor_tensor(out=ycen2_sb, in0=ycen_sb, in1=ycen_sb, op=OP.mult)
    ycen_2x = singles.tile([128, 1], F32)
    V.tensor_scalar(ycen_2x, ycen_sb, 2.0, None, OP.mult)
    ycen_p8 = singles.tile([128, 1], F32)
    V.tensor_scalar(ycen_p8, ycen_sb, 8.0, None, OP.add)
    ycen_m8 = singles.tile([128, 1], F32)
    V.tensor_scalar(ycen_m8, ycen_sb, -8.0, None, OP.add)

    # global per-gaussian planes, [128, chunk]-vectorized
    w6L = singles.tile([128, NLC, 8], F32)   # w0..w5 global planes (low)
    w6H = singles.tile([128, NHC, 8], F32)   # (high)
    f2g = singles.tile([128, NHC, G], F32)   # global phase constants
    swg = singles.tile([128, NHC], F32)      # sum_g wg per gaussian
    c3 = singles.tile([128, NCH, 3], BF16)
    diag = singles.tile([128, NHC * G * 128], BF16)
    modsb = singles.tile([128, NHC, SB], BF16)
    fsl = singles.tile([128, NHC, G, 2], F32)   # phase slope planes [fx,fy]/2pi

    # ---------------- per-gaussian prep ----------------
    with tc.tile_pool(name="prep", bufs=1) as prep, \
         tc.tile_pool(name="prep_ps", bufs=2, space="PSUM") as prep_ps:

        nc.gpsimd.dma_start(out=basis_sb, in_=basis[:])
        nc.gpsimd.dma_start(out=basisq_sb, in_=basisq[:])

        def prep_group(nch, c0, w6, mu_d, ch_d, ft_d, op_d):
            mu_t = prep.tile([128, 2, nch], F32, name=f"mu{c0}")
            nc.gpsimd.dma_start(out=mu_t, in_=mu_d[:].rearrange("(c p) k -> p k c", p=128))
            ch_t = prep.tile([128, 3, nch], F32, name=f"ch{c0}")
            nc.gpsimd.dma_start(out=ch_t, in_=ch_d[:].rearrange("(c p) k -> p k c", p=128))
            ft_t = prep.tile([128, 3, nch], F32, name=f"ft{c0}")
            nc.gpsimd.dma_start(out=ft_t, in_=ft_d[:].rearrange("(c p) k -> p k c", p=128))
            op_t = prep.tile([128, 1, nch], F32, name=f"op{c0}")
            nc.gpsimd.dma_start(out=op_t, in_=op_d[:].rearrange("(c p) k -> p k c", p=128))

            m_t = prep.tile([128, 2, nch], F32, name=f"m{c0}")
            S.activation(m_t, mu_t, AF.Tanh)
            xci = prep.tile([128, nch], F32, name=f"xci{c0}")
            V.tensor_scalar(xci, m_t[:, 0, :], 128.0, None, OP.mult)
            yci = prep.tile([128, nch], F32, name=f"yci{c0}")
            V.tensor_scalar(yci, m_t[:, 1, :], 128.0, None, OP.mult)

            l1 = prep.tile([128, nch], F32, name=f"l1{c0}")
            V.tensor_scalar(l1, ch_t[:, 0, :], 0.5, None, OP.add)
            l2 = ch_t[:, 1, :]
            l3 = prep.tile([128, nch], F32, name=f"l3{c0}")
            V.tensor_scalar(l3, ch_t[:, 2, :], 0.5, None, OP.add)
            sxx = prep.tile([128, nch], F32, name=f"sxx{c0}")
            V.tensor_tensor(out=sxx, in0=l1, in1=l1, op=OP.mult)
            sxy = prep.tile([128, nch], F32, name=f"sxy{c0}")
            V.tensor_tensor(out=sxy, in0=l1, in1=l2, op=OP.mult)
            syy = prep.tile([128, nch], F32, name=f"syy{c0}")
            V.tensor_tensor(out=syy, in0=l2, in1=l2, op=OP.mult)
            t2 = prep.tile([128, nch], F32, name=f"t2{c0}")
            V.tensor_tensor(out=t2, in0=l3, in1=l3, op=OP.mult)
            V.tensor_tensor(out=syy, in0=syy, in1=t2, op=OP.add)
            det = prep.tile([128, nch], F32, name=f"det{c0}")
            V.tensor_tensor(out=det, in0=sxx, in1=syy, op=OP.mult)
            V.tensor_tensor(out=t2, in0=sxy, in1=sxy, op=OP.mult)
            V.tensor_tensor(out=det, in0=det, in1=t2, op=OP.subtract)
            inv = prep.tile([128, nch], F32, name=f"inv{c0}")
            V.reciprocal(inv, det)
            A = prep.tile([128, nch], F32, name=f"A{c0}")
            V.tensor_tensor(out=A, in0=syy, in1=inv, op=OP.mult)
            C = prep.tile([128, nch], F32, name=f"C{c0}")
            V.tensor_tensor(out=C, in0=sxx, in1=inv, op=OP.mult)
            NB = prep.tile([128, nch], F32, name=f"NB{c0}")   # -B
            V.tensor_tensor(out=NB, in0=sxy, in1=inv, op=OP.mult)

            # global sigma planes: w0=A/2, w1=B, w2=C/2,
            # w3=-(A xci + B yci), w4=-(B xci + C yci), w5=sigma at (0,0)
            V.tensor_scalar(w6[:, :, 0], A, 0.5, None, OP.mult)
            V.tensor_scalar(w6[:, :, 1], NB, -1.0, None, OP.mult)
            V.tensor_scalar(w6[:, :, 2], C, 0.5, None, OP.mult)
            ta = prep.tile([128, nch], F32, name=f"ta{c0}")
            tb = prep.tile([128, nch], F32, name=f"tb{c0}")
            V.tensor_tensor(out=ta, in0=NB, in1=yci, op=OP.mult)
            V.tensor_tensor(out=tb, in0=A, in1=xci, op=OP.mult)
            V.tensor_tensor(out=w6[:, :, 3], in0=ta, in1=tb, op=OP.subtract)
            V.tensor_tensor(out=ta, in0=NB, in1=xci, op=OP.mult)
            V.tensor_tensor(out=tb, in0=C, in1=yci, op=OP.mult)
            V.tensor_tensor(out=w6[:, :, 4], in0=ta, in1=tb, op=OP.subtract)
            V.tensor_tensor(out=ta, in0=xci, in1=w6[:, :, 3], op=OP.mult)
            V.tensor_tensor(out=tb, in0=yci, in1=w6[:, :, 4], op=OP.mult)
            V.tensor_tensor(out=ta, in0=ta, in1=tb, op=OP.add)
            V.tensor_scalar(w6[:, :, 5], ta, -0.5, None, OP.mult)

            # funnel DMA'd tiles through DVE copies: downstream DVE ops then
            # depend only on same-engine results (no extra semaphore waits)
            ftc = prep.tile([128, 3, nch], F32, name=f"ftc{c0}")
            V.tensor_copy(out=ftc, in_=ft_t)
            opc = prep.tile([128, nch], F32, name=f"opc{c0}")
            V.tensor_copy(out=opc, in_=op_t[:, 0, :])
            colf = prep.tile([128, 3, nch], F32, name=f"colf{c0}")
            for kk in range(3):
                V.tensor_tensor(out=colf[:, kk, :], in0=ftc[:, kk, :],
                                in1=opc, op=OP.mult)
            V.tensor_copy(out=c3[:, c0:c0 + nch, :].rearrange("p c k -> p k c"),
                          in_=colf)
            return xci, yci

        prep_group(NLC, 0, w6L, lmu, lch, lft, lop)
        xci_h, yci_h = prep_group(NHC, NLC, w6H, hmu, hch, hft, hop)

        # global bf16 hi/lo splits of the quadratic weight planes (for the
        # split-operand K=13 sigma matmul that sidesteps f32r's ~11-bit
        # mantissa: products of hi parts are exact, cross terms are small)
        for key, nch, w6 in (("L", NLC, w6L), ("H", NHC, w6H)):
            hi = singles.tile([128, nch, 3], BF16, name=f"hi{key}")
            lo = singles.tile([128, nch, 3], F32, name=f"lo{key}")
            for j in range(3):
                V.tensor_copy(out=hi[:, :, j], in_=w6[:, :, j])
                V.tensor_tensor(out=lo[:, :, j], in0=w6[:, :, j],
                                in1=hi[:, :, j], op=OP.subtract)
            if key == "L":
                hiL, loL = hi, lo
            else:
                hiH, loH = hi, lo
        whiL, wloL, whiH, wloH = hiL, loL, hiH, loH

        fx_d = prep.tile([128, G, NHC], F32)
        nc.gpsimd.dma_start(out=fx_d, in_=gfx[:].rearrange("(c p) g -> p g c", p=128))
        fy_d = prep.tile([128, G, NHC], F32)
        nc.gpsimd.dma_start(out=fy_d, in_=gfy[:].rearrange("(c p) g -> p g c", p=128))
        wg_d = prep.tile([128, G, NHC], F32)
        nc.gpsimd.dma_start(out=wg_d, in_=gwg[:].rearrange("(c p) g -> p g c", p=128))
        fx_t = prep.tile([128, G, NHC], F32)
        V.tensor_copy(out=fx_t, in_=fx_d)
        fy_t = prep.tile([128, G, NHC], F32)
        V.tensor_copy(out=fy_t, in_=fy_d)
        wg_t = prep.tile([128, G, NHC], F32)
        V.tensor_copy(out=wg_t, in_=wg_d)

        # phase slope planes [fx/2pi, fy/2pi] and global constant
        # f2g = TOFF - (fx*xci + fy*yci)/2pi
        pa = prep.tile([128, NHC], F32)
        pb = prep.tile([128, NHC], F32)
        for g in range(G):
            V.tensor_scalar(fsl[:, :, g, 0], fx_t[:, g, :], INV2PI, None, OP.mult)
            V.tensor_scalar(fsl[:, :, g, 1], fy_t[:, g, :], INV2PI, None, OP.mult)
            V.tensor_tensor(out=pa, in0=fx_t[:, g, :], in1=xci_h, op=OP.mult)
            V.tensor_tensor(out=pb, in0=fy_t[:, g, :], in1=yci_h, op=OP.mult)
            V.tensor_tensor(out=pa, in0=pa, in1=pb, op=OP.add)
            V.tensor_scalar(f2g[:, :, g], pa, -INV2PI, None, OP.mult)

        # diag(-2*wg) blocks for the half-angle carrier sum, and swg = sum_g wg
        wgm2 = prep.tile([128, G, NHC], F32)
        V.tensor_scalar(wgm2, wg_t, -2.0, None, OP.mult)
        V.tensor_tensor(out=swg, in0=wg_t[:, 0, :], in1=wg_t[:, 1, :], op=OP.add)
        V.tensor_tensor(out=swg, in0=swg, in1=wg_t[:, 2, :], op=OP.add)
        V.tensor_tensor(out=swg, in0=swg, in1=wg_t[:, 3, :], op=OP.add)
        for c in range(NHC):
            for g in range(G):
                V.tensor_tensor(
                    out=diag[:, (c * G + g) * 128:(c * G + g + 1) * 128],
                    in0=ident_sb,
                    in1=wgm2[:, g, c:c + 1].to_broadcast([128, 128]),
                    op=OP.mult)

    # ---------------- main loop over column blocks ----------------
    tc.strict_bb_all_engine_barrier()
    with tc.tile_pool(name="quad", bufs=2, space="PSUM") as quad, \
         tc.tile_pool(name="modp", bufs=1, space="PSUM") as modp, \
         tc.tile_pool(name="imgp", bufs=1, space="PSUM") as imgp, \
         tc.tile_pool(name="wrk", bufs=3) as wrk, \
         tc.tile_pool(name="spool", bufs=2) as spool, \
         tc.tile_pool(name="s2pool", bufs=2) as s2pool, \
         tc.tile_pool(name="sbw", bufs=2) as sbw, \
         tc.tile_pool(name="outp", bufs=2) as outp:

        for sb in range(NSB):
            bs = sb * SB
            x0 = _x0(sb)

            # --- per-block sigma weight planes (w0..w4 recentered, -w5') ---
            # w3' = w3 + 2*x0*w0 + y0*w1 ; w4' = w4 + x0*w1 + 2*y0*w2
            # w5' = w5 + x0*w3 + y0*w4 + x0^2*w0 + x0*y0*w1 + y0^2*w2
            wp = {}
            nw5 = {}
            for key, nch, w6 in (("L", NLC, w6L), ("H", NHC, w6H)):
                wploc = sbw.tile([128, nch, 8], F32, name=f"wp{key}", tag=f"wp{key}")
                for j in range(3):
                    V.tensor_copy(out=wploc[:, :, j], in_=w6[:, :, j])
                tmp = sbw.tile([128, nch], F32, name=f"tmp{key}", tag=f"tm{key}")
                V.scalar_tensor_tensor(out=tmp, in0=w6[:, :, 0], scalar=2.0 * x0,
                                       in1=w6[:, :, 3], op0=OP.mult, op1=OP.add)
                V.scalar_tensor_tensor(out=wploc[:, :, 3], in0=w6[:, :, 1],
                                       scalar=ycen_sb, in1=tmp,
                                       op0=OP.mult, op1=OP.add)
                V.scalar_tensor_tensor(out=tmp, in0=w6[:, :, 1], scalar=x0,
                                       in1=w6[:, :, 4], op0=OP.mult, op1=OP.add)
                V.scalar_tensor_tensor(out=wploc[:, :, 4], in0=w6[:, :, 2],
                                       scalar=ycen_2x, in1=tmp,
                                       op0=OP.mult, op1=OP.add)
                # -w5' accumulation
                n5 = sbw.tile([128, nch], F32, name=f"n5{key}", tag=f"n5{key}")
                V.scalar_tensor_tensor(out=n5, in0=w6[:, :, 3], scalar=x0,
                                       in1=w6[:, :, 5], op0=OP.mult, op1=OP.add)
                V.scalar_tensor_tensor(out=n5, in0=w6[:, :, 0], scalar=x0 * x0,
                                       in1=n5, op0=OP.mult, op1=OP.add)
                V.scalar_tensor_tensor(out=n5, in0=w6[:, :, 4], scalar=ycen_sb,
                                       in1=n5, op0=OP.mult, op1=OP.add)
                V.tensor_scalar(tmp, w6[:, :, 1], x0, None, OP.mult)
                V.scalar_tensor_tensor(out=n5, in0=tmp, scalar=ycen_sb,
                                       in1=n5, op0=OP.mult, op1=OP.add)
                V.scalar_tensor_tensor(out=n5, in0=w6[:, :, 2], scalar=ycen2_sb,
                                       in1=n5, op0=OP.mult, op1=OP.add)
                V.tensor_scalar(n5, n5, -1.0, None, OP.mult)
                wp[key] = wploc
                nw5[key] = n5

            # assemble split 13-row weight planes and transpose -> g5t f32r
            # rows: [w0h,w0h,w0l, w1h,w1h,w1l, w2h,w2h,w2l, w3h,w3l, w4h,w4l]
            # matching basis rows [x2h,x2l,x2h, xyh,xyl,xyh, y2h,y2l,y2h,
            # xc,xc, yc,yc]
            wq = {}
            for key, nch, whi, wlo in (("L", NLC, whiL, wloL),
                                       ("H", NHC, whiH, wloH)):
                wqt = sbw.tile([128, nch, 16], F32, name=f"wq{key}", tag=f"wq{key}")
                for j in range(3):
                    V.tensor_copy(
                        out=wqt[:, :, 3 * j:3 * j + 2],
                        in_=whi[:, :, j:j + 1].to_broadcast([128, nch, 2]))
                    V.tensor_copy(out=wqt[:, :, 3 * j + 2], in_=wlo[:, :, j])
                for j, base in ((3, 9), (4, 11)):
                    hh = sbw.tile([128, nch], BF16, name=f"hh{key}{j}",
                                  tag=f"hh{key}{j}")
                    V.tensor_copy(out=hh, in_=wp[key][:, :, j])
                    V.tensor_copy(out=wqt[:, :, base], in_=hh)
                    V.tensor_tensor(out=wqt[:, :, base + 1],
                                    in0=wp[key][:, :, j], in1=hh, op=OP.subtract)
                wq[key] = wqt
            g5t = sbw.tile([13, NCH * 128], F32R, name="g5t", tag="g5t")
            for q in range(NCH // 8):
                tp5 = quad.tile([13, 1024], F32, name="tp5", tag="quad")
                for j in range(8):
                    c = q * 8 + j
                    key, cl = ("L", c) if c < NLC else ("H", c - NLC)
                    T.transpose(tp5[:, j * 128:(j + 1) * 128],
                                wq[key][:, cl, 0:13], ident_sb)
                V.tensor_copy(out=g5t[:, q * 1024:(q + 1) * 1024], in_=tp5)

            # phase weight planes for this block, with per-16x16-quarter
            # rounded integer offsets: rows [f0, f1, fq(q=0..3)] where
            # fq = (f2g + xq*f0 + yq*f1) - round(same). quarter q = 2*xh + yh.
            MAGIC = 1.5 * 2 ** 23
            fpl = sbw.tile([128, NHC, G, 8], F32, name="fpl", tag="fpl")
            fbt = sbw.tile([128, NHC], F32, name="fbt", tag="fbt")
            fbk = sbw.tile([128, NHC], F32, name="fbk", tag="fbk")
            fbb = sbw.tile([128, NHC], F32, name="fbb", tag="fbb")
            for g in range(G):
                V.tensor_copy(out=fpl[:, :, g, 0], in_=fsl[:, :, g, 0])
                V.tensor_copy(out=fpl[:, :, g, 1], in_=fsl[:, :, g, 1])
                # block-center constant fbb = f2g + x0*f0 + y0*f1
                V.scalar_tensor_tensor(out=fbb, in0=fsl[:, :, g, 0],
                                       scalar=x0, in1=f2g[:, :, g],
                                       op0=OP.mult, op1=OP.add)
                V.scalar_tensor_tensor(out=fbb, in0=fsl[:, :, g, 1],
                                       scalar=ycen_sb, in1=fbb,
                                       op0=OP.mult, op1=OP.add)
                for q in range(4):
                    xq = x0 + (8.0 if q >= 2 else -8.0)
                    yq = ycen_p8 if (q % 2) else ycen_m8
                    # quarter-center value (used only for the integer offset)
                    V.scalar_tensor_tensor(out=fbt, in0=fsl[:, :, g, 0],
                                           scalar=xq, in1=f2g[:, :, g],
                                           op0=OP.mult, op1=OP.add)
                    V.scalar_tensor_tensor(out=fbt, in0=fsl[:, :, g, 1],
                                           scalar=yq, in1=fbt,
                                           op0=OP.mult, op1=OP.add)
                    V.tensor_scalar(fbk, fbt, MAGIC, MAGIC, OP.add, OP.subtract)
                    V.tensor_tensor(out=fpl[:, :, g, 2 + q], in0=fbb, in1=fbk,
                                    op=OP.subtract)
            # transpose to lhsT layout fT[6, (hc*G+g)*128]
            fT = sbw.tile([6, NHC * G * 128], F32R, name="fT", tag="fT")
            for hc in range(NHC):
                tpF = quad.tile([6, G * 128], F32, name="tpF", tag="quad")
                for g in range(G):
                    T.transpose(tpF[:, g * 128:(g + 1) * 128],
                                fpl[:, hc, g, 0:6], ident_sb)
                V.tensor_copy(out=fT[:, hc * G * 128:(hc + 1) * G * 128], in_=tpF)

            # ---- SIN phase (half-angle: cos(p) = 1 - 2 sin^2(p/2)) ----
            for hc in range(NHC):
                mod_ps = modp.tile([128, SB], F32, name="mod_ps", tag="mod")
                for g in range(G):
                    t_ps = quad.tile([128, SB], F32, name="t_ps", tag="quad")
                    for h in range(2):
                        T.matmul(
                            t_ps[:, h * 512:(h + 1) * 512],
                            fT[:, (hc * G + g) * 128:(hc * G + g + 1) * 128],
                            basisq_sb[:, bs + h * 512:bs + (h + 1) * 512],
                            start=True, stop=True)
                    sg = spool.tile([128, SB], F32, name="sg")
                    S.activation(sg, t_ps, AF.Sin, scale=math.pi)
                    s2 = s2pool.tile([128, SB], BF16, name="s2")
                    V.tensor_tensor(out=s2, in0=sg, in1=sg, op=OP.mult)
                    for h in range(2):
                        T.matmul(
                            mod_ps[:, h * 512:(h + 1) * 512],
                            diag[:, (hc * G + g) * 128:(hc * G + g + 1) * 128],
                            s2[:, h * 512:(h + 1) * 512],
                            start=(g == 0), stop=(g == G - 1))
                V.tensor_copy(out=modsb[:, hc, :], in_=mod_ps)

            # ---- EXP phase ----
            img_ps = imgp.tile([3, SB], F32, name="img_ps", tag="img")
            for c in range(NCH):
                key, cl = ("L", c) if c < NLC else ("H", c - NLC)
                sig_ps = quad.tile([128, SB], F32, name="sig_ps", tag="quad")
                for h in range(2):
                    T.matmul(
                        sig_ps[:, h * 512:(h + 1) * 512],
                        g5t[:, c * 128:(c + 1) * 128],
                        basis_sb[:, bs + h * 512:bs + (h + 1) * 512],
                        start=True, stop=True)
                w = wrk.tile([128, SB], BF16, name="w", tag="w")
                if c < NLC:
                    S.activation(w, sig_ps, AF.Exp, bias=nw5[key][:, cl:cl + 1],
                                 scale=-1.0)
                else:
                    env = wrk.tile([128, SB], BF16, name="env", tag="env")
                    S.activation(env, sig_ps, AF.Exp, bias=nw5[key][:, cl:cl + 1],
                                 scale=-1.0)
                    V.scalar_tensor_tensor(out=w, in0=modsb[:, cl, :],
                                           scalar=swg[:, cl:cl + 1], in1=env,
                                           op0=OP.add, op1=OP.mult)
                for h in range(2):
                    T.matmul(
                        img_ps[:, h * 512:(h + 1) * 512],
                        c3[:, c, :],
                        w[:, h * 512:(h + 1) * 512],
                        start=(c == 0), stop=(c == NCH - 1))

            outt = outp.tile([3, SB], F32, name="outt")
            V.tensor_scalar(outt, img_ps, 0.0, 1.0, OP.max, OP.min)
            nc.gpsimd.dma_start(out=out_ext[:, bs:bs + SB], in_=outt)


def _host_inputs(low_mu, high_mu, low_chol, high_chol, low_feat, high_feat,
                 low_opac, high_opac, gabor_freqs, gabor_weights):
    """Pure-layout host prep: reshapes, constant bases, per-core slicing."""
    fx = np.ascontiguousarray(gabor_freqs[:, 0].reshape(NH, G))
    fy = np.ascontiguousarray(gabor_freqs[:, 1].reshape(NH, G))
    wg = np.ascontiguousarray(gabor_weights[:, 0].reshape(NH, G))
    ident = np.eye(128, dtype=np.float32)

    common = {
        "lmu": np.ascontiguousarray(low_mu, np.float32),
        "lch": np.ascontiguousarray(low_chol, np.float32),
        "lft": np.ascontiguousarray(low_feat, np.float32),
        "lop": np.ascontiguousarray(low_opac, np.float32),
        "hmu": np.ascontiguousarray(high_mu, np.float32),
        "hch": np.ascontiguousarray(high_chol, np.float32),
        "hft": np.ascontiguousarray(high_feat, np.float32),
        "hop": np.ascontiguousarray(high_opac, np.float32),
        "gfx": fx.astype(np.float32), "gfy": fy.astype(np.float32),
        "gwg": wg.astype(np.float32),
        "ident": ident,
    }

    in_maps = []
    for k in range(NCORES):
        y0 = 32.0 * k - 112.0
        cols = []
        colsq = []
        for sbi in range(NSB):
            x0 = _x0(sbi)
            xs = np.arange(sbi * CB, (sbi + 1) * CB, dtype=np.float32) + 0.5 - 128.0 - x0
            ys = np.arange(k * ROWS, (k + 1) * ROWS, dtype=np.float32) + 0.5 - 128.0 - y0
            YC, XC = np.meshgrid(ys, xs, indexing="ij")
            xc, yc = XC.ravel(), YC.ravel()   # y-major within block

            def _bf16(v):
                u = np.asarray(v, np.float32).view(np.uint32)
                return (((u + 0x8000 + ((u >> 16) & 1)) & 0xFFFF0000)
                        .astype(np.uint32)).view(np.float32)
            x2h = _bf16(xc * xc); x2l = xc * xc - x2h
            xyh = _bf16(xc * yc); xyl = xc * yc - xyh
            y2h = _bf16(yc * yc); y2l = yc * yc - y2h
            cols.append(np.stack([x2h, x2l, x2h, xyh, xyl, xyh,
                                  y2h, y2l, y2h, xc, xc, yc, yc], 0))
            # quarter indicators: q = 2*(xc >= 0) + (yc >= 0); 16x16 quarters
            qsel = (2 * (xc >= 0) + (yc >= 0)).astype(np.int64)
            qrows = np.zeros((4, xc.size), np.float32)
            qrows[qsel, np.arange(xc.size)] = 1.0
            colsq.append(np.concatenate([np.stack([xc, yc], 0), qrows], 0))
        basis = np.concatenate(cols, axis=1).astype(np.float32)    # [13, 8192]
        basisq = np.concatenate(colsq, axis=1).astype(np.float32)  # [6, 8192]
        m = dict(common)
        m["basis"] = np.ascontiguousarray(basis)
        m["basisq"] = np.ascontiguousarray(basisq)
        m["ycen"] = np.full((128, 1), y0, np.float32)
        in_maps.append(m)
    return in_maps


def _assemble(results):
    """Reassemble per-core column-block outputs into [1,3,256,256]."""
    img = np.zeros((3, H, W), np.float32)
    for k in range(NCORES):
        o = np.asarray(results[k]["out"]).reshape(3, NSB, ROWS, CB)
        img[:, k * ROWS:(k + 1) * ROWS, :] = o.transpose(0, 2, 1, 3).reshape(
            3, ROWS, W)
    return img[None]


def kernel(**inputs):
    inputs = {k: np.asarray(v, np.float32) for k, v in inputs.items()}
    if "nc" not in _CACHE:
        _CACHE["nc"] = _build_program()
    nc = _CACHE["nc"]
    in_maps = _host_inputs(**inputs)
    res = run_bass_kernel_spmd(nc, in_maps, list(range(NCORES)))
    return _assemble(res.results).astype(np.float32)


if __name__ == "__main__":
    import reference
    ins = {k: np.asarray(v) for k, v in reference.setup_inputs().items()}
    out = kernel(**ins)
    ref = np.asarray(reference.reference(**reference.setup_inputs()))
    rel = np.linalg.norm(out - ref) / np.linalg.norm(ref)
    print("Relative error:", rel)

